# revision 23
# baseline (speedup 1.0000x reference)
"""Grouped per-expert SwiGLU FFN (MoE experts) on 8 TRN2 NeuronCores.

Reference computation (per expert e):
    h1  = x[e] @ w1[e]          # [T, H]
    h3  = x[e] @ w3[e]          # [T, H]
    mid = silu(h1) * h3         # [T, H]
    out = mid @ w2[e].T         # [T, D]

Sharding: expert-parallel, one expert per core (E == n_cores == 8).
No collectives needed; outputs concatenate along E.

Per-core kernel (T=256, D=2048, H=8192), bf16 compute / fp32 accumulate.
The kernel is memory-bound (96 MiB of bf16 weights per core vs ~280us of
PE work), so every weight tensor is pre-tiled on the host into the exact
per-DMA-chunk layout: each dma_start reads one fully contiguous 2 MiB
block with 16 KiB per-partition lines (vs 0.5-4 KiB strided lines for
the natural layouts).

  Phase 1 (up/gate): for each 512-wide slice of H, accumulate over D
    h1T/h3T [128h, 256t] in PSUM (lhsT = w1/w3 tile [128d, 128h],
    rhs = xt tile [128d, 256t]; the (LDWEIGHTS; N=256 matmul) pair
    measures ~84ns -- the fastest per-FLOP pattern on this HW), then
    midT[:, ht, :] = silu(h1T) * h3T (bf16, kept in SBUF).
  Phase 2 (down): d-slice-outer, h-inner: for each 256-wide slice of D,
    sweep the full H contraction (lhsT = midT sub-tile [128h, 128t],
    fresh per matmul; rhs = w2t tile [128h, 256d]) accumulating
    out[t-tile, d-slice] in one PSUM bank per t-tile; drain each bank as
    its d-slice finishes, overlapped with the next slice's matmuls.
    (N=256 matmuls measure ~84-103ns vs ~244ns for N=512 -- the entire
    phase is then DMA-bound, not PE-bound.)

Overlap details: xt and the first w1/w3 slices are split into sub-DMAs
across both HWDGE rings so the PE starts ~4us earlier; two w2t chunks
are prefetched during phase 1 to hide the phase transition (phase-1 DMA
runs at ~343 of the ~348 GB/s 8-core-contended per-core rate, so there
is slack for exactly ~2 chunks); each d-slice's drain is emitted
t-major so it overlaps the next slice's matmuls; the output is returned
as bf16 (the f32 cast happens on the host) to halve the output DMA.

Measured (reps-differenced, 8 cores concurrent): best 307.6us, median
~338us over 6 runs (run-to-run variance is +-13%, HBM co-tenancy).  The
PE-pure floor of this instruction stream is ~296us and the 8-core DMA
floor ~284us, so the best draws sit ~4% off roofline.
"""

import sys

if "/opt/trn_rl_repo" not in sys.path:
    sys.path.insert(0, "/opt/trn_rl_repo")

import numpy as np
import ml_dtypes

import concourse.mybir as mybir
import concourse.tile as tile
from concourse import bacc
from concourse.bass_utils import run_bass_kernel_spmd

E, T, D, H = 8, 256, 2048, 8192
P = 128
KD = D // P          # 16 d-tiles (contraction tiles for up/gate proj)
HT = H // P          # 64 h-tiles
TT = T // P          # 2 t-tiles
HS = H // 512        # 16 w1/w3 h-slices (512 wide)
HCH = 32             # h-tiles per w2t DMA chunk (2 MiB)
NCH = (HT // HCH) * (D // 256)   # 16 w2t chunks (2 per d-slice)
DS = D // 256        # 8 output d-slices (256 wide)

BF16 = mybir.dt.bfloat16
F32 = mybir.dt.float32

_CACHED = {}


def _build(reps: int = 1):
    nc = bacc.Bacc("TRN2", target_bir_lowering=False, debug=False)
    xt_d = nc.dram_tensor("xt", [P, KD, T], BF16, kind="ExternalInput").ap()
    w1_d = nc.dram_tensor("w1", [HS, P, KD, 512], BF16, kind="ExternalInput").ap()
    w3_d = nc.dram_tensor("w3", [HS, P, KD, 512], BF16, kind="ExternalInput").ap()
    w2t_d = nc.dram_tensor("w2t", [NCH, P, HCH, 256], BF16,
                           kind="ExternalInput").ap()
    out_d = nc.dram_tensor("out", [TT, DS, P, 256], BF16,
                           kind="ExternalOutput").ap()

    with tile.TileContext(nc) as tc:
        with tc.tile_pool(name="persist", bufs=1) as cpool, \
             tc.tile_pool(name="w2pool", bufs=4) as w2pool:
          for _rep in range(reps):
            xt_sb = cpool.tile([P, KD, T], BF16, tag="xt", name="xt_sb")
            midT = cpool.tile([P, HT, T], BF16, tag="midT", name="midT")

            # xt split across both rings for fast startup
            nc.sync.dma_start(xt_sb[:, 0:8, :], xt_d[:, 0:8, :])
            nc.scalar.dma_start(xt_sb[:, 8:16, :], xt_d[:, 8:16, :])

            w2_tiles = {}

            # ---- Phase 1: up/gate projections + SwiGLU -> midT ----
            with (
                tc.tile_pool(name="wpool", bufs=3) as wpool,
                tc.tile_pool(name="act", bufs=3) as apool,
                tc.tile_pool(name="ps1", bufs=4, space="PSUM") as ps1,
            ):
                for j in range(HS):
                    w1_sb = wpool.tile([P, KD, 512], BF16, tag="w1", name="w1_sb")
                    w3_sb = wpool.tile([P, KD, 512], BF16, tag="w3", name="w3_sb")
                    if j == 0:
                        nc.sync.dma_start(w1_sb[:, 0:4, :], w1_d[j, :, 0:4, :])
                        nc.sync.dma_start(w1_sb[:, 4:10, :], w1_d[j, :, 4:10, :])
                        nc.sync.dma_start(w1_sb[:, 10:16, :], w1_d[j, :, 10:16, :])
                        nc.scalar.dma_start(w3_sb[:, 0:4, :], w3_d[j, :, 0:4, :])
                        nc.scalar.dma_start(w3_sb[:, 4:10, :], w3_d[j, :, 4:10, :])
                        nc.scalar.dma_start(w3_sb[:, 10:16, :], w3_d[j, :, 10:16, :])
                    else:
                        nc.sync.dma_start(w1_sb, w1_d[j])
                        nc.scalar.dma_start(w3_sb, w3_d[j])
                    for s in range(4):
                        ht = j * 4 + s
                        ssl = slice(s * P, (s + 1) * P)
                        # h1 and h3 share one PSUM bank: the first chain's
                        # start=True clears the whole bank's has_written
                        # bits, so the second chain opens with start=False
                        # (its first matmul overwrites) -- halves the
                        # per-chain bank-clear cost.  h3 runs FIRST so the
                        # bank's last PE write is h1's stop: the silu/mul
                        # reads can then never overlap a PE write to this
                        # bank (PSUM bank collisions are hardware-fatal).
                        hb_ps = ps1.tile([P, 2 * T], F32, tag="hb", name="hb_ps")
                        h1_ps = hb_ps[:, 0:T]
                        h3_ps = hb_ps[:, T:2 * T]
                        for kd in range(KD):
                            nc.tensor.matmul(
                                h3_ps,
                                w3_sb[:, kd, ssl],
                                xt_sb[:, kd, :],
                                start=(kd == 0),
                                stop=(kd == KD - 1),
                            )
                        for kd in range(KD):
                            nc.tensor.matmul(
                                h1_ps,
                                w1_sb[:, kd, ssl],
                                xt_sb[:, kd, :],
                                start=False,
                                stop=(kd == KD - 1),
                                skip_group_check=True,
                            )
                        silu_sb = apool.tile([P, T], F32, tag="silu", name="silu_sb")
                        nc.scalar.activation(
                            silu_sb, h1_ps, mybir.ActivationFunctionType.Silu
                        )
                        nc.vector.tensor_mul(
                            out=midT[:, ht, :], in0=silu_sb, in1=h3_ps
                        )
                    if j == 2:
                        # prefetch first two w2t chunks behind the w1/w3
                        # queue (phase-1 DMA slack fits ~2 chunks)
                        for hc in range(2):
                            w2_sb = w2pool.tile([P, HCH, 256], BF16, tag="w2",
                                                name="w2_sb")
                            eng = nc.sync if hc % 2 == 0 else nc.scalar
                            eng.dma_start(w2_sb, w2t_d[hc])
                            w2_tiles[hc] = w2_sb

            # ---- Phase 2: down projection (d-slice outer, h inner) ----
            with (
                tc.tile_pool(name="opool", bufs=3) as opool,
                tc.tile_pool(name="ps2", bufs=2, space="PSUM") as ps2,
            ):
                for dsl in range(DS):
                    o_ps = [ps2.tile([P, 256], F32, tag=f"o{t}", name=f"o_ps{t}")
                            for t in range(TT)]
                    for c in range(2):
                        hc = dsl * 2 + c
                        if hc in w2_tiles:
                            w2_sb = w2_tiles.pop(hc)
                        else:
                            w2_sb = w2pool.tile([P, HCH, 256], BF16, tag="w2",
                                                name="w2_sb")
                            eng = nc.sync if hc % 2 == 0 else nc.scalar
                            eng.dma_start(w2_sb, w2t_d[hc])
                        if c == 0:
                            for hh in range(HCH):
                                for t in range(TT):
                                    tsl = slice(t * P, (t + 1) * P)
                                    nc.tensor.matmul(
                                        o_ps[t], midT[:, hh, tsl],
                                        w2_sb[:, hh, :],
                                        start=(hh == 0), stop=False,
                                    )
                        else:
                            # second half-chunk t-major so t0's drain
                            # overlaps t1's remaining matmuls
                            for t in range(TT):
                                tsl = slice(t * P, (t + 1) * P)
                                for hh in range(HCH):
                                    nc.tensor.matmul(
                                        o_ps[t], midT[:, HCH + hh, tsl],
                                        w2_sb[:, hh, :],
                                        start=False, stop=(hh == HCH - 1),
                                    )
                                o_sb = opool.tile([P, 256], BF16, tag="osb",
                                                  name="o_sb")
                                nc.any.tensor_copy(out=o_sb, in_=o_ps[t])
                                nc.sync.dma_start(out_d[t, dsl], o_sb)

    nc.compile()
    return nc


def _get_nc():
    if "nc" not in _CACHED:
        _CACHED["nc"] = _build()
    return _CACHED["nc"]


def _pack(x_e, w1_e, w2_e, w3_e):
    bf = ml_dtypes.bfloat16
    # xt[p, kd, t] = x[t, kd*128+p]
    xt = np.ascontiguousarray(
        np.asarray(x_e).T.reshape(KD, P, T).transpose(1, 0, 2)).astype(bf)
    # w1t[hs, p, kd, h] = w1[kd*128+p, hs*512+h]
    w1t = np.ascontiguousarray(
        np.asarray(w1_e).reshape(KD, P, HS, 512).transpose(2, 1, 0, 3)).astype(bf)
    w3t = np.ascontiguousarray(
        np.asarray(w3_e).reshape(KD, P, HS, 512).transpose(2, 1, 0, 3)).astype(bf)
    # w2t[dsl*2+c, p, hh, dd] = w2[dsl*256+dd, (c*32+hh)*128+p]
    w2t = np.ascontiguousarray(
        np.asarray(w2_e).T.reshape(2, HCH, P, DS, 256)
        .transpose(3, 0, 2, 1, 4).reshape(NCH, P, HCH, 256)).astype(bf)
    return {"xt": xt, "w1": w1t, "w3": w3t, "w2t": w2t}


def kernel(x, w1, w2, w3, **_unused):
    """x: [E,T,D] f32; w1,w2,w3: [E,D,H] f32 -> [E,T,D] f32."""
    in_maps = [_pack(x[e], w1[e], w2[e], w3[e]) for e in range(E)]
    nc = _get_nc()
    res = run_bass_kernel_spmd(nc, in_maps, core_ids=list(range(E)))
    # out_d is [TT, DS, P, 256]: t = tt*128 + p, d = dsl*256 + dd
    out = np.stack(
        [
            res.results[e]["out"].transpose(0, 2, 1, 3).reshape(T, D)
            for e in range(E)
        ],
        axis=0,
    )
    return out.astype(np.float32, copy=False)
